# revision 12
# baseline (speedup 1.0000x reference)
"""BertImageSelfAttention Trainium2 kernel.

Shapes (fixed): hidden_states [4, 2048, 1024], 16 heads x 64, text [4, 64, 768].
Sharding: 8 cores = 4 batches x 2 head-groups (8 heads each). Each core computes
its batch's attention context for its 8 heads; host reassembles [4, 2048, 1024].

Per-core device pipeline (all matmuls bf16 with fp32 PSUM accumulation). The
scalar engine's Exp over the full S x S scores (33.5M elem/core at 1 elem/
cycle/lane) is the pacing engine, so the kernel is organized to saturate it:

  A. pooled text -> dynamic Q/K gates (tiny matmuls + sigmoid).
  B. loads: gate-critical tensors on the HWDGE queue first; x^T split into
     four s-slices on the SWDGE queue so projections start mid-load.
  C. Q/K projections for head-pair 0 run first; attention head 0 starts as
     soon as its first K/Q blocks land (~15us in). The V projection and the
     remaining Q/K e-chunks are dosed into the tensor engine's slack inside
     later heads' attention loops, underneath the Exp-bound steady state.
  D. per head: S^T tiles = K^T.T @ Q^T (keys on partitions), ACT Exp with
     scale=1/8 and per-partition bias=attention_mask, ctx^T[65,512] +=
     Vaug.T @ E^T over 16 key chunks (row 64 = softmax denominator).
  E. per (s-half, head): evict ctx^T to SBUF, DMA-gather the [1,1024]
     denominator row across 128 partitions, one cheap [128,8] reciprocal,
     DMA scatter+broadcast back to [64,1024], fused multiply + bias add,
     DMA out [64,1024] fp32 (output stays ctx^T; host transposes).
"""

import os

import numpy as np
import ml_dtypes

import concourse.bass as bass
import concourse.bacc as bacc
import concourse.tile as tile
from concourse import mybir
from concourse.bass_utils import run_bass_kernel_spmd

P = 128
B, S, DV = 4, 2048, 1024
H, Dh = 16, 64
T, DT = 64, 768
NCORES = 8
E = 512          # head-group width (8 heads x 64)
CC = DV // P     # 8 contraction chunks for projections
ECH = E // P     # 4 e-chunks
DC = DT // P     # 6 text-dim chunks
SC = S // P      # 16 seq chunks of 128
SBL = S // 512   # 4 seq blocks of 512
HPC = 8          # heads per core

FP32 = mybir.dt.float32
BF16 = mybir.dt.bfloat16
AF = mybir.ActivationFunctionType
OP = mybir.AluOpType

BF16_NP = ml_dtypes.bfloat16

_CACHE = {}

# module-level stash of the last BassKernelResults (for test.py introspection)
last_results = None


def _emit(tc, aps):
    nc = tc.nc
    xT = aps["xT"].rearrange("(c p) s -> p c s", p=P)          # [128, 8, 2048]
    wq = aps["wq"].rearrange("(c p) e -> p c e", p=P)          # [128, 8, 512]
    wk = aps["wk"].rearrange("(c p) e -> p c e", p=P)
    wv = aps["wv"].rearrange("(c p) e -> p c e", p=P)
    wdq = aps["wdq"].rearrange("(c p) e -> p c e", p=P)        # [128, 6, 512]
    wdk = aps["wdk"].rearrange("(c p) e -> p c e", p=P)
    txt = aps["txt"]                                           # [64, 768] bf16
    tmask = aps["tmask"]                                       # [64, 1] bf16
    amask = aps["amask"].rearrange("(c p) -> p c", p=P)        # [128, 16]
    bq = aps["bq"].rearrange("(c p) -> p c", p=P)              # [128, 4]
    bk = aps["bk"].rearrange("(c p) -> p c", p=P)
    bdq = aps["bdq"].rearrange("(c p) -> p c", p=P)
    bdk = aps["bdk"].rearrange("(c p) -> p c", p=P)
    bv = aps["bv"]                                             # [512]
    out = aps["out"]                                           # [8, 64, 2048] f32

    from contextlib import ExitStack

    with ExitStack() as ctx:
        wpool = ctx.enter_context(tc.tile_pool(name="wpool", bufs=1))
        xpool = ctx.enter_context(tc.tile_pool(name="xpool", bufs=1))
        qkpool = ctx.enter_context(tc.tile_pool(name="qkpool", bufs=1))
        vpool = ctx.enter_context(tc.tile_pool(name="vpool", bufs=1))
        etp = ctx.enter_context(tc.tile_pool(name="etp", bufs=4))
        rbp = ctx.enter_context(tc.tile_pool(name="rbp", bufs=2))
        outp = ctx.enter_context(tc.tile_pool(name="outp", bufs=2))
        smallp = ctx.enter_context(tc.tile_pool(name="smallp", bufs=1))
        rcp = ctx.enter_context(tc.tile_pool(name="rcp", bufs=2))
        # PSUM: 8 banks = scp 2 x [128,1024] (2 banks each) + accp 2 x 1 bank
        # (ctx accumulators) + pjp 2 x 1 bank (projection scratch, dosed into
        # the attention loop — separate pool so rotation can't land on a
        # live ctx accumulator).
        accp = ctx.enter_context(tc.tile_pool(name="accp", bufs=2, space="PSUM"))
        pjp = ctx.enter_context(tc.tile_pool(name="pjp", bufs=2, space="PSUM"))
        scp = ctx.enter_context(tc.tile_pool(name="scp", bufs=2, space="PSUM"))

        # ---- gate-critical small loads (HWDGE queue) ----
        # text tensors padded to 128 partitions (zero rows 64..127) so every
        # matmul runs in uniform (128,128) PE tile mode — no mode switches.
        txt_sb = smallp.tile([P, DT], BF16, tag="txt")
        nc.vector.memset(txt_sb[T:P, :], 0.0)
        nc.sync.dma_start(out=txt_sb[0:T, :], in_=txt)
        # mask as a [128,128] stationary: column 0 = mask, rest zero -> M=128
        tmask_sb = smallp.tile([P, P], BF16, tag="tmask")
        nc.vector.memset(tmask_sb, 0.0)
        nc.sync.dma_start(out=tmask_sb[0:T, 0:1], in_=tmask)
        ones_sb = smallp.tile([P, 1], BF16, tag="ones")
        nc.vector.memset(ones_sb, 1.0)
        onesf_sb = smallp.tile([1, 1], FP32, tag="onesf")
        nc.vector.memset(onesf_sb, 1.0)
        onesf64 = smallp.tile([1, Dh], FP32, tag="onesf64")
        nc.vector.memset(onesf64, 1.0)
        amask_sb = smallp.tile([P, SC], FP32, tag="amask")
        nc.sync.dma_start(out=amask_sb, in_=amask)
        bq_sb = smallp.tile([P, ECH], FP32, tag="bq")
        nc.sync.dma_start(out=bq_sb, in_=bq)
        bk_sb = smallp.tile([P, ECH], FP32, tag="bk")
        nc.sync.dma_start(out=bk_sb, in_=bk)
        bdq_sb = smallp.tile([P, ECH], FP32, tag="bdq")
        nc.sync.dma_start(out=bdq_sb, in_=bdq)
        bdk_sb = smallp.tile([P, ECH], FP32, tag="bdk")
        nc.sync.dma_start(out=bdk_sb, in_=bdk)
        bvT_sb = smallp.tile([P, ECH], FP32, tag="bvT")
        nc.sync.dma_start(out=bvT_sb, in_=bv.rearrange("(c p) -> p c", p=P))
        bdq_row = smallp.tile([1, E], FP32, tag="bdqr")
        nc.sync.dma_start(out=bdq_row, in_=aps["bdq"].rearrange("(o e) -> o e", o=1))
        bdk_row = smallp.tile([1, E], FP32, tag="bdkr")
        nc.sync.dma_start(out=bdk_row, in_=aps["bdk"].rearrange("(o e) -> o e", o=1))

        wdq_sb = wpool.tile([P, DC, E], BF16, tag="wdq")
        nc.sync.dma_start(out=wdq_sb, in_=wdq)
        wdk_sb = wpool.tile([P, DC, E], BF16, tag="wdk")
        nc.sync.dma_start(out=wdk_sb, in_=wdk)

        # ---- big loads, spread across the HWDGE queues ----
        # All triggers are fire-and-forget (fresh tiles, no WAR waits), so
        # four queues stream concurrently and the preload finishes in
        # ~bytes/BW instead of serializing behind one queue. ec0's inputs
        # (wq/wk + early xT slices) land first; wv/late slices trail.
        xT_sb = xpool.tile([P, CC, S], BF16, tag="xT")
        wv_sb = wpool.tile([P, CC, E], BF16, tag="wv")
        wq_sb = wpool.tile([P, CC, E], BF16, tag="wq")
        wk_sb = wpool.tile([P, CC, E], BF16, tag="wk")
        nc.scalar.dma_start(out=wq_sb, in_=wq)
        nc.scalar.dma_start(out=wk_sb, in_=wk)
        nc.gpsimd.dma_start(out=xT_sb[:, :, 0:512], in_=xT[:, :, 0:512])
        nc.gpsimd.dma_start(out=wv_sb, in_=wv)
        for ss in range(1, SBL):
            sl = slice(ss * 512, (ss + 1) * 512)
            nc.gpsimd.dma_start(out=xT_sb[:, :, sl], in_=xT[:, :, sl])

        # ---- phase A: pooled text + gates ----
        # pr row 0, cols 0:768 = sum_t txt[t,:]*mask[t]; col 768 = sum_t mask[t]
        pr = scp.tile([P, 1024], FP32, tag="sc")
        nc.tensor.matmul(pr[:, 0:512], lhsT=tmask_sb, rhs=txt_sb[:, 0:512],
                         start=True, stop=True)
        nc.tensor.matmul(pr[:, 512:768], lhsT=tmask_sb, rhs=txt_sb[:, 512:768],
                         start=True, stop=True)
        nc.tensor.matmul(pr[:, 768:769], lhsT=tmask_sb, rhs=ones_sb,
                         start=True, stop=True)
        rmsum = smallp.tile([1, 1], FP32, tag="rmsum")
        nc.vector.reciprocal(rmsum, pr[0:1, 768:769])
        prow = smallp.tile([1, DT], BF16, tag="prow")
        nc.vector.tensor_scalar(prow, pr[0:1, 0:768], rmsum, None, OP.mult)

        # scatter pooled row -> poolT [128, 6] (dt on partitions) via tiny
        # SBUF->SBUF DMA (dt = c*128 + p)
        poolT = smallp.tile([P, DC], BF16, tag="poolT")
        for c in range(DC):
            nc.sync.dma_start(
                out=poolT[:, c:c + 1],
                in_=prow[0:1, c * P:(c + 1) * P],
            )

        # gates: g = 1 + sigmoid(pool @ Wd + bd); also g*b for fused bias.
        # Row orientation: out [1,512] = poolT.T @ Wd accumulated over the 6
        # dt-chunks, bias folded via a K=1 matmul against a bd row, one
        # [1,512] sigmoid, then a partition-scatter DMA to [128,4].
        gq_sb = smallp.tile([P, ECH], FP32, tag="gq")
        gk_sb = smallp.tile([P, ECH], FP32, tag="gk")
        gbq_sb = smallp.tile([P, ECH], FP32, tag="gbq")
        gbk_sb = smallp.tile([P, ECH], FP32, tag="gbk")
        for (nm, wd_sb, bdrow, b_sb, g_sb, gb_sb) in (
            ("q", wdq_sb, bdq_row, bq_sb, gq_sb, gbq_sb),
            ("k", wdk_sb, bdk_row, bk_sb, gk_sb, gbk_sb),
        ):
            gp = pjp.tile([P, 512], FP32, tag="acc", name=f"g{nm}")
            for c in range(DC):
                nc.tensor.matmul(
                    gp[0:1, :],
                    lhsT=poolT[:, c:c + 1],
                    rhs=wd_sb[:, c, :],
                    start=(c == 0), stop=False,
                )
            nc.tensor.matmul(gp[0:1, :], lhsT=onesf_sb, rhs=bdrow,
                             start=False, stop=True)
            grow = smallp.tile([1, E], FP32, tag=f"grow{nm}")
            nc.scalar.activation(grow, gp[0:1, :], AF.Sigmoid)
            # scatter g row -> [128,4] matching the "(c p) -> p c" bias layout
            for c in range(ECH):
                nc.sync.dma_start(out=g_sb[:, c:c + 1],
                                  in_=grow[0:1, c * P:(c + 1) * P])
            nc.vector.tensor_scalar(g_sb, g_sb, 1.0, None, OP.add)
            nc.vector.tensor_mul(gb_sb, g_sb, b_sb)

        # ---- projection emitters (dosed into the attention loop) ----
        QT = qkpool.tile([P, ECH, S], BF16, tag="QT")
        KTp = qkpool.tile([P, HPC, S], BF16, tag="KTp")
        nc.gpsimd.memset(KTp, 0.0)
        Vaug = vpool.tile([P, SC, HPC, Dh + 1], BF16, tag="Vaug")

        def emit_qk_half(ec, ss, which):
            """One projection half-group: 8 matmuls + eviction for Q or K."""
            sl = slice(ss * 512, (ss + 1) * 512)
            if which == "q":
                ps = pjp.tile([P, 512], FP32, tag="acc", name=f"psq{ec}_{ss}")
                for c in range(CC):
                    nc.tensor.matmul(
                        ps,
                        lhsT=wq_sb[:, c, ec * P:(ec + 1) * P],
                        rhs=xT_sb[:, c, sl],
                        start=(c == 0), stop=(c == CC - 1),
                    )
                # (x@W)*g + g*b fused into eviction, cast bf16
                nc.vector.tensor_scalar(
                    QT[:, ec, sl], ps,
                    gq_sb[:, ec:ec + 1], gbq_sb[:, ec:ec + 1],
                    OP.mult, OP.add,
                )
            else:
                psk = pjp.tile([P, 512], FP32, tag="acc", name=f"psk{ec}_{ss}")
                for c in range(CC):
                    nc.tensor.matmul(
                        psk,
                        lhsT=wk_sb[:, c, ec * P:(ec + 1) * P],
                        rhs=xT_sb[:, c, sl],
                        start=(c == 0), stop=(c == CC - 1),
                    )
                # K^T per-head zero-padded to 128 partitions (head h real on
                # partitions (h%2)*64..) so score matmuls contract K=128 in
                # the same (128,128) mode as everything else.
                for hi in range(2):
                    pp = slice(hi * Dh, (hi + 1) * Dh)
                    nc.vector.tensor_scalar(
                        KTp[pp, 2 * ec + hi, sl], psk[pp, :],
                        gk_sb[pp, ec:ec + 1], gbk_sb[pp, ec:ec + 1],
                        OP.mult, OP.add,
                    )

        def emit_v(t):
            """V projection t-chunk: 8 matmuls + eviction into Vaug."""
            ps = pjp.tile([P, 512], FP32, tag="acc", name=f"psv{t}")
            for c in range(CC):
                nc.tensor.matmul(
                    ps,
                    lhsT=xT_sb[:, c, t * P:(t + 1) * P],
                    rhs=wv_sb[:, c, :],
                    start=(c == 0), stop=(c == CC - 1),
                )
            nc.vector.tensor_copy(
                Vaug[:, t, :, 0:Dh],
                ps.rearrange("p (h d) -> p h d", h=HPC),
            )
            nc.vector.memset(Vaug[:, t, :, Dh:Dh + 1], 1.0)

        # head-pair 0's projections run up front, ordered so the first
        # score tile's inputs (Q ss0, Q ss1, K ss0) complete first.
        for ss, which in ((0, "q"), (1, "q"), (0, "k"), (1, "k"),
                          (2, "k"), (3, "k"), (2, "q"), (3, "q")):
            emit_qk_half(0, ss, which)

        # Remaining projections are dosed into attention t-iterations:
        # dose[(h, sp, t)] = emitters to run before that tile's score
        # matmuls. h0/sp0 hosts V (ctx t needs Vaug[t] that same iter);
        # h1, h3, h5 host the ec1..ec3 half-groups, one per 4 iters.
        dose = {}
        for t in range(SC):
            dose[(0, 0, t)] = [lambda t=t: emit_v(t)]
        for ec in (1, 2, 3):
            h = 2 * ec - 1
            for k in range(2 * SBL):
                sp, t = divmod(4 * k + 2, SC)
                ss, which = k // 2, ("q", "k")[k % 2]
                dose.setdefault((h, sp, t), []).append(
                    lambda ec=ec, ss=ss, w=which: emit_qk_half(ec, ss, w))

        # ---- phases D+E: attention ----
        # output stays in ctx^T layout [head, d, s]; host transposes to [s, e]
        for h in range(HPC):             # heads sequential, uniform PE mode
            hp, hi = h // 2, h % 2
            for sp in range(2):          # s-half: columns sp*1024 .. +1024
                ctx_ps = [accp.tile([P, 512], FP32, tag="acc",
                                    name=f"ctx{sp}_{h}_{k}") for k in range(2)]
                for t in range(SC):
                    for fn in dose.get((h, sp, t), ()):
                        fn()
                    sps = scp.tile([P, 1024], FP32, tag="sc")
                    for j in range(2):
                        s0 = sp * 1024 + j * 512
                        nc.tensor.matmul(
                            sps[:, j * 512:(j + 1) * 512],
                            lhsT=KTp[:, h, t * P:(t + 1) * P],
                            rhs=QT[:, hp, s0:s0 + 512],
                            start=True, stop=True,
                        )
                    et = etp.tile([P, 1024], BF16, tag="et")
                    nc.scalar.activation(et, sps, AF.Exp,
                                         bias=amask_sb[:, t:t + 1],
                                         scale=0.125)
                    for j in range(2):
                        nc.tensor.matmul(
                            ctx_ps[j][0:Dh + 1, :],
                            lhsT=Vaug[:, t, h, :],
                            rhs=et[:, j * 512:(j + 1) * 512],
                            start=(t == 0), stop=(t == SC - 1),
                        )
                # phase E: normalize in ctx^T layout (no PE transposes):
                # out[d, s] = ctx_unnorm[d, s] * (1/denom[s]) + bv[h*64+d].
                # Evacuate PSUM immediately so the banks free early.
                cs = rbp.tile([Dh + 1, 1024], FP32, tag="cs")
                for j in range(2):
                    nc.vector.tensor_copy(
                        cs[:, j * 512:(j + 1) * 512], ctx_ps[j][0:Dh + 1, :])
                # denominator row [1,1024] -> [128,8] across partitions, one
                # cheap reciprocal, then scatter back + partition-broadcast.
                rpack = rcp.tile([P, 8], FP32, tag="rpack")
                nc.sync.dma_start(out=rpack, in_=cs[Dh:Dh + 1, :])
                nc.vector.reciprocal(rpack, rpack)
                rrow = rcp.tile([1, 1024], FP32, tag="rrow")
                nc.sync.dma_start(out=rrow, in_=rpack)
                # partition-broadcast via a tiny PE outer product
                # (ones[64] x rrow) instead of a single-partition-read DMA
                # (which runs at ~24 GB/s and dominates the kernel tail).
                for j in range(2):
                    rcb_ps = pjp.tile([P, 512], FP32, tag="acc",
                                      name=f"rcb{sp}_{h}_{j}")
                    nc.tensor.matmul(
                        rcb_ps[0:Dh, :], lhsT=onesf64,
                        rhs=rrow[:, j * 512:(j + 1) * 512],
                        start=True, stop=True,
                    )
                    ot = outp.tile([Dh, 512], FP32, tag="outsb")
                    nc.vector.tensor_mul(
                        ot, cs[0:Dh, j * 512:(j + 1) * 512], rcb_ps[0:Dh, :])
                    nc.vector.tensor_scalar(
                        ot, ot,
                        bvT_sb[hi * Dh:(hi + 1) * Dh, hp:hp + 1], None,
                        OP.add,
                    )
                    # out DMA on the SWDGE queue: idle post-load, so its
                    # sem waits never back up the sync queue
                    nc.gpsimd.dma_start(
                        out=out[h, :, sp * 1024 + j * 512:
                                sp * 1024 + (j + 1) * 512], in_=ot)


def _build():
    key = "nc"
    if key in _CACHE:
        return _CACHE[key]
    nc = bacc.Bacc("TRN2", target_bir_lowering=False, debug=False,
                   enable_asserts=False)
    aps = {}

    def din(name, shape, dt):
        aps[name] = nc.dram_tensor(name, shape, dt, kind="ExternalInput").ap()

    din("xT", [DV, S], BF16)
    din("wq", [DV, E], BF16)
    din("wk", [DV, E], BF16)
    din("wv", [DV, E], BF16)
    din("wdq", [DT, E], BF16)
    din("wdk", [DT, E], BF16)
    din("txt", [T, DT], BF16)
    din("tmask", [T, 1], BF16)
    din("amask", [S], FP32)
    din("bq", [E], FP32)
    din("bk", [E], FP32)
    din("bv", [E], FP32)
    din("bdq", [E], FP32)
    din("bdk", [E], FP32)
    aps["out"] = nc.dram_tensor("out", [HPC, Dh, S], FP32,
                                kind="ExternalOutput").ap()

    with tile.TileContext(nc) as tc:
        _emit(tc, aps)
    nc.compile()
    _CACHE[key] = nc
    return nc


def kernel(**inputs):
    global last_results
    hs = np.asarray(inputs["hidden_states"], dtype=np.float32)
    amask = np.asarray(inputs["attention_mask"], dtype=np.float32)
    txt = np.asarray(inputs["txt_embedding"], dtype=np.float32)
    tmask = np.asarray(inputs["txt_attention_mask"], dtype=np.float32)
    Wq = np.asarray(inputs["Wq"], dtype=np.float32)
    Wk = np.asarray(inputs["Wk"], dtype=np.float32)
    Wv = np.asarray(inputs["Wv"], dtype=np.float32)
    Wdq = np.asarray(inputs["Wdq"], dtype=np.float32)
    Wdk = np.asarray(inputs["Wdk"], dtype=np.float32)
    bq = np.asarray(inputs["bq"], dtype=np.float32)
    bk = np.asarray(inputs["bk"], dtype=np.float32)
    bv = np.asarray(inputs["bv"], dtype=np.float32)
    bdq = np.asarray(inputs["bdq"], dtype=np.float32)
    bdk = np.asarray(inputs["bdk"], dtype=np.float32)

    nc = _build()

    in_maps = []
    for c in range(NCORES):
        b, g = c // 2, c % 2
        cols = slice(g * E, (g + 1) * E)
        in_maps.append({
            "xT": np.ascontiguousarray(hs[b].T).astype(BF16_NP),
            "wq": Wq[:, cols].astype(BF16_NP),
            "wk": Wk[:, cols].astype(BF16_NP),
            "wv": Wv[:, cols].astype(BF16_NP),
            "wdq": Wdq[:, cols].astype(BF16_NP),
            "wdk": Wdk[:, cols].astype(BF16_NP),
            "txt": txt[b].astype(BF16_NP),
            "tmask": tmask[b].astype(BF16_NP),
            "amask": np.ascontiguousarray(amask[b, 0, 0]),
            "bq": np.ascontiguousarray(bq[cols]),
            "bk": np.ascontiguousarray(bk[cols]),
            "bv": np.ascontiguousarray(bv[cols]),
            "bdq": np.ascontiguousarray(bdq[cols]),
            "bdk": np.ascontiguousarray(bdk[cols]),
        })

    tr = int(os.environ.get("BASS_KERNEL_TRACE", "0"))
    if tr == 2:
        # warm the NEFF (compile+load+run untraced), then trace a second run
        run_bass_kernel_spmd(nc, in_maps, list(range(NCORES)), trace=False)
    res = run_bass_kernel_spmd(nc, in_maps, list(range(NCORES)), trace=bool(tr))
    last_results = res

    outp = np.empty((B, S, DV), dtype=np.float32)
    for c in range(NCORES):
        b, g = c // 2, c % 2
        # device output is ctx^T [head, d, s] -> [s, head*64+d]
        co = res.results[c]["out"].transpose(2, 0, 1).reshape(S, E)
        outp[b, :, g * E:(g + 1) * E] = co
    return outp


# revision 17
# speedup vs baseline: 1.1876x; 1.1876x over previous
"""BertImageSelfAttention Trainium2 kernel.

Shapes (fixed): hidden_states [4, 2048, 1024], 16 heads x 64, text [4, 64, 768].
Sharding: 8 cores = 4 batches x 2 head-groups (8 heads each). Each core computes
its batch's attention context for its 8 heads; host reassembles [4, 2048, 1024].

Per-core device pipeline (all matmuls bf16 with fp32 PSUM accumulation). The
scalar engine's Exp over the full S x S scores (33.5M elem/core at 1 elem/
cycle/lane) is the pacing engine, so the kernel is organized to saturate it:

  A. pooled text -> dynamic Q/K gates (tiny matmuls + sigmoid).
  B. loads: gate-critical tensors on the HWDGE queue first; x^T split into
     four s-slices on the SWDGE queue so projections start mid-load.
  C. Q/K projections for head-pair 0 run first; attention head 0 starts as
     soon as its first K/Q blocks land (~15us in). The V projection and the
     remaining Q/K e-chunks are dosed into the tensor engine's slack inside
     later heads' attention loops, underneath the Exp-bound steady state.
  D. per head: S^T tiles = K^T.T @ Q^T (keys on partitions), ACT Exp with
     scale=1/8 and per-partition bias=attention_mask, ctx^T[65,512] +=
     Vaug.T @ E^T over 16 key chunks (row 64 = softmax denominator).
  E. per (s-half, head): evict ctx^T to SBUF, DMA-gather the [1,1024]
     denominator row across 128 partitions, one cheap [128,8] reciprocal,
     DMA scatter+broadcast back to [64,1024], fused multiply + bias add,
     DMA out [64,1024] fp32 (output stays ctx^T; host transposes).
"""

import os

import numpy as np
import ml_dtypes

import concourse.bass as bass
import concourse.bacc as bacc
import concourse.tile as tile
from concourse import mybir
from concourse.bass_utils import run_bass_kernel_spmd

P = 128
B, S, DV = 4, 2048, 1024
H, Dh = 16, 64
T, DT = 64, 768
NCORES = 8
E = 512          # head-group width (8 heads x 64)
CC = DV // P     # 8 contraction chunks for projections
ECH = E // P     # 4 e-chunks
DC = DT // P     # 6 text-dim chunks
SC = S // P      # 16 seq chunks of 128
SBL = S // 512   # 4 seq blocks of 512
HPC = 8          # heads per core

FP32 = mybir.dt.float32
BF16 = mybir.dt.bfloat16
AF = mybir.ActivationFunctionType
OP = mybir.AluOpType

BF16_NP = ml_dtypes.bfloat16

_CACHE = {}

# module-level stash of the last BassKernelResults (for test.py introspection)
last_results = None


def _emit(tc, aps):
    nc = tc.nc
    xT = aps["xT"].rearrange("(c p) s -> p c s", p=P)          # [128, 8, 2048]
    wq = aps["wq"].rearrange("(c p) e -> p c e", p=P)          # [128, 8, 512]
    wk = aps["wk"].rearrange("(c p) e -> p c e", p=P)
    wv = aps["wv"].rearrange("(c p) e -> p c e", p=P)
    wdq = aps["wdq"].rearrange("(c p) e -> p c e", p=P)        # [128, 6, 512]
    wdk = aps["wdk"].rearrange("(c p) e -> p c e", p=P)
    txt = aps["txt"]                                           # [64, 768] bf16
    tmask = aps["tmask"]                                       # [64, 1] bf16
    amask = aps["amask"]                                       # [128, 16]
    bq = aps["bq"]                                             # [128, 4]
    bk = aps["bk"]
    bdq = aps["bdq"]
    bdk = aps["bdk"]
    bv = aps["bv"]                                             # [128, 4]
    out = aps["out"]                                           # [8, 64, 2048] f32

    from contextlib import ExitStack

    with ExitStack() as ctx:
        wpool = ctx.enter_context(tc.tile_pool(name="wpool", bufs=1))
        xpool = ctx.enter_context(tc.tile_pool(name="xpool", bufs=1))
        qkpool = ctx.enter_context(tc.tile_pool(name="qkpool", bufs=1))
        vpool = ctx.enter_context(tc.tile_pool(name="vpool", bufs=1))
        etp = ctx.enter_context(tc.tile_pool(name="etp", bufs=4))
        rbp = ctx.enter_context(tc.tile_pool(name="rbp", bufs=2))
        outp = ctx.enter_context(tc.tile_pool(name="outp", bufs=2))
        smallp = ctx.enter_context(tc.tile_pool(name="smallp", bufs=1))
        rcp = ctx.enter_context(tc.tile_pool(name="rcp", bufs=2))
        # PSUM: 8 banks = scp 2 x [128,1024] (2 banks each) + accp 2 x 1 bank
        # (ctx accumulators) + pjp 2 x 1 bank (projection scratch, dosed into
        # the attention loop — separate pool so rotation can't land on a
        # live ctx accumulator).
        accp = ctx.enter_context(tc.tile_pool(name="accp", bufs=2, space="PSUM"))
        pjp = ctx.enter_context(tc.tile_pool(name="pjp", bufs=2, space="PSUM"))
        scp = ctx.enter_context(tc.tile_pool(name="scp", bufs=2, space="PSUM"))

        # ---- gate-critical small loads (HWDGE queue) ----
        # text tensors padded to 128 partitions (zero rows 64..127) so every
        # matmul runs in uniform (128,128) PE tile mode — no mode switches.
        txt_sb = smallp.tile([P, DT], BF16, tag="txt")
        nc.vector.memset(txt_sb[T:P, :], 0.0)
        nc.sync.dma_start(out=txt_sb[0:T, :], in_=txt)
        # mask as a [128,128] stationary: column 0 = mask, rest zero -> M=128
        tmask_sb = smallp.tile([P, P], BF16, tag="tmask")
        nc.vector.memset(tmask_sb, 0.0)
        nc.sync.dma_start(out=tmask_sb[0:T, 0:1], in_=tmask)
        ones_sb = smallp.tile([P, 1], BF16, tag="ones")
        nc.vector.memset(ones_sb, 1.0)
        onesf64 = smallp.tile([1, Dh], FP32, tag="onesf64")
        nc.vector.memset(onesf64, 1.0)
        amask_sb = smallp.tile([P, SC], FP32, tag="amask")
        nc.sync.dma_start(out=amask_sb, in_=amask)
        bq_sb = smallp.tile([P, ECH], FP32, tag="bq")
        nc.sync.dma_start(out=bq_sb, in_=bq)
        bk_sb = smallp.tile([P, ECH], FP32, tag="bk")
        nc.sync.dma_start(out=bk_sb, in_=bk)
        bdq_sb = smallp.tile([P, ECH], FP32, tag="bdq")
        nc.sync.dma_start(out=bdq_sb, in_=bdq)
        bdk_sb = smallp.tile([P, ECH], FP32, tag="bdk")
        nc.sync.dma_start(out=bdk_sb, in_=bdk)
        bvT_sb = smallp.tile([P, ECH], FP32, tag="bvT")
        nc.sync.dma_start(out=bvT_sb, in_=bv)

        wdq_sb = wpool.tile([P, DC, E], BF16, tag="wdq")
        nc.sync.dma_start(out=wdq_sb, in_=wdq)
        wdk_sb = wpool.tile([P, DC, E], BF16, tag="wdk")
        nc.sync.dma_start(out=wdk_sb, in_=wdk)

        # ---- big loads, spread across the HWDGE queues ----
        # All triggers are fire-and-forget (fresh tiles, no WAR waits), so
        # four queues stream concurrently and the preload finishes in
        # ~bytes/BW instead of serializing behind one queue. ec0's inputs
        # (wq/wk + early xT slices) land first; wv/late slices trail.
        xT_sb = xpool.tile([P, CC, S], BF16, tag="xT")
        wv_sb = wpool.tile([P, CC, E], BF16, tag="wv")
        wq_sb = wpool.tile([P, CC, E], BF16, tag="wq")
        wk_sb = wpool.tile([P, CC, E], BF16, tag="wk")
        nc.scalar.dma_start(out=wq_sb, in_=wq)
        nc.scalar.dma_start(out=wk_sb, in_=wk)
        nc.gpsimd.dma_start(out=xT_sb[:, :, 0:512], in_=xT[:, :, 0:512])
        nc.gpsimd.dma_start(out=wv_sb, in_=wv)
        for ss in range(1, SBL):
            sl = slice(ss * 512, (ss + 1) * 512)
            nc.gpsimd.dma_start(out=xT_sb[:, :, sl], in_=xT[:, :, sl])

        # ---- phase A: pooled text + gates ----
        # poolT [128,6] (dt on partitions) computed DIRECTLY: col c =
        # txt_chunk_c.T @ tmask (no row-scatter DMAs). Col 6 row 0 holds
        # sum(mask); its reciprocal is partition-broadcast once and folded
        # into the sigmoid's per-partition scale (pool stays unnormalized).
        pps = pjp.tile([P, 512], FP32, tag="acc", name="poolps")
        for c in range(DC):
            nc.tensor.matmul(pps[:, c:c + 1],
                             lhsT=txt_sb[:, c * P:(c + 1) * P],
                             rhs=tmask_sb[:, 0:1], start=True, stop=True)
        nc.tensor.matmul(pps[:, DC:DC + 1], lhsT=tmask_sb, rhs=ones_sb,
                         start=True, stop=True)
        rmsum = smallp.tile([1, 1], FP32, tag="rmsum")
        nc.vector.reciprocal(rmsum, pps[0:1, DC:DC + 1])
        rmsumb = smallp.tile([P, 1], FP32, tag="rmsumb")
        rm_bcast = bass.AP(
            tensor=rmsum.tensor, offset=rmsum.offset,
            ap=[list(rmsum.ap[0]), [0, P]] + [list(d) for d in rmsum.ap[1:]],
        )
        nc.sync.dma_start(out=rmsumb, in_=rm_bcast)
        poolT = smallp.tile([P, DC], BF16, tag="poolT")
        nc.vector.tensor_copy(poolT, pps[:, 0:DC])

        # gates: g = 1 + sigmoid((pool_u @ Wd) * rmsum + bd); also g*b
        gq_sb = smallp.tile([P, ECH], FP32, tag="gq")
        gk_sb = smallp.tile([P, ECH], FP32, tag="gk")
        gbq_sb = smallp.tile([P, ECH], FP32, tag="gbq")
        gbk_sb = smallp.tile([P, ECH], FP32, tag="gbk")
        for (nm, wd_sb, bd_sb, b_sb, g_sb, gb_sb) in (
            ("q", wdq_sb, bdq_sb, bq_sb, gq_sb, gbq_sb),
            ("k", wdk_sb, bdk_sb, bk_sb, gk_sb, gbk_sb),
        ):
            for ec in range(ECH):
                gp = pjp.tile([P, 512], FP32, tag="acc", name=f"g{nm}{ec}")
                for c in range(DC):
                    nc.tensor.matmul(
                        gp[:, 0:1],
                        lhsT=wd_sb[:, c, ec * P:(ec + 1) * P],
                        rhs=poolT[:, c:c + 1],
                        start=(c == 0), stop=(c == DC - 1),
                    )
                nc.scalar.activation(g_sb[:, ec:ec + 1], gp[:, 0:1], AF.Sigmoid,
                                     bias=bd_sb[:, ec:ec + 1], scale=rmsumb)
            nc.vector.tensor_scalar(g_sb, g_sb, 1.0, None, OP.add)
            nc.vector.tensor_mul(gb_sb, g_sb, b_sb)

        # ---- projection emitters (dosed into the attention loop) ----
        QT = qkpool.tile([P, ECH, S], BF16, tag="QT")
        KTp = qkpool.tile([P, HPC, S], BF16, tag="KTp")
        nc.gpsimd.memset(KTp, 0.0)
        Vaug = vpool.tile([P, SC, HPC, Dh + 1], BF16, tag="Vaug")

        def emit_qk_half(ec, ss, which):
            """One projection half-group: 8 matmuls + eviction for Q or K."""
            sl = slice(ss * 512, (ss + 1) * 512)
            if which == "q":
                ps = pjp.tile([P, 512], FP32, tag="acc", name=f"psq{ec}_{ss}")
                for c in range(CC):
                    nc.tensor.matmul(
                        ps,
                        lhsT=wq_sb[:, c, ec * P:(ec + 1) * P],
                        rhs=xT_sb[:, c, sl],
                        start=(c == 0), stop=(c == CC - 1),
                    )
                # (x@W)*g + g*b fused into eviction, cast bf16
                nc.vector.tensor_scalar(
                    QT[:, ec, sl], ps,
                    gq_sb[:, ec:ec + 1], gbq_sb[:, ec:ec + 1],
                    OP.mult, OP.add,
                )
            else:
                psk = pjp.tile([P, 512], FP32, tag="acc", name=f"psk{ec}_{ss}")
                for c in range(CC):
                    nc.tensor.matmul(
                        psk,
                        lhsT=wk_sb[:, c, ec * P:(ec + 1) * P],
                        rhs=xT_sb[:, c, sl],
                        start=(c == 0), stop=(c == CC - 1),
                    )
                # K^T per-head zero-padded to 128 partitions (head h real on
                # partitions (h%2)*64..) so score matmuls contract K=128 in
                # the same (128,128) mode as everything else.
                for hi in range(2):
                    pp = slice(hi * Dh, (hi + 1) * Dh)
                    nc.vector.tensor_scalar(
                        KTp[pp, 2 * ec + hi, sl], psk[pp, :],
                        gk_sb[pp, ec:ec + 1], gbk_sb[pp, ec:ec + 1],
                        OP.mult, OP.add,
                    )

        def emit_v(t):
            """V projection t-chunk: 8 matmuls + eviction into Vaug."""
            ps = pjp.tile([P, 512], FP32, tag="acc", name=f"psv{t}")
            for c in range(CC):
                nc.tensor.matmul(
                    ps,
                    lhsT=xT_sb[:, c, t * P:(t + 1) * P],
                    rhs=wv_sb[:, c, :],
                    start=(c == 0), stop=(c == CC - 1),
                )
            nc.vector.tensor_copy(
                Vaug[:, t, :, 0:Dh],
                ps.rearrange("p (h d) -> p h d", h=HPC),
            )
            nc.vector.memset(Vaug[:, t, :, Dh:Dh + 1], 1.0)

        # head-pair 0's projections run up front, ordered so the first
        # score tile's inputs (Q ss0, Q ss1, K ss0) complete first.
        for ss, which in ((0, "q"), (1, "q"), (0, "k"), (1, "k"),
                          (2, "k"), (3, "k"), (2, "q"), (3, "q")):
            emit_qk_half(0, ss, which)

        # Remaining projections are dosed into attention t-iterations:
        # dose[(h, sp, t)] = emitters to run before that tile's score
        # matmuls. h0/sp0 hosts V (ctx t needs Vaug[t] that same iter);
        # h1, h3, h5 host the ec1..ec3 half-groups, one per 4 iters.
        dose = {}
        for t in range(SC):
            dose[(0, 0, t)] = [lambda t=t: emit_v(t)]
        for ec in (1, 2, 3):
            h = 2 * ec - 1
            for k in range(2 * SBL):
                sp, t = divmod(4 * k + 2, SC)
                ss, which = k // 2, ("q", "k")[k % 2]
                dose.setdefault((h, sp, t), []).append(
                    lambda ec=ec, ss=ss, w=which: emit_qk_half(ec, ss, w))

        # ---- phases D+E: attention ----
        # output stays in ctx^T layout [head, d, s]; host transposes to [s, e]
        def make_phase_e(h, sp, cs):
            hp, hi = h // 2, h % 2

            def run():
                # denominator row [1,1024] -> [128,8] across partitions, one
                # cheap reciprocal, scatter back to a row, then partition-
                # broadcast via a tiny PE outer product (ones[64] x rrow).
                rpack = rcp.tile([P, 8], FP32, tag="rpack")
                nc.sync.dma_start(out=rpack, in_=cs[Dh:Dh + 1, :])
                nc.vector.reciprocal(rpack, rpack)
                rrow = rcp.tile([1, 1024], FP32, tag="rrow")
                nc.sync.dma_start(out=rrow, in_=rpack)
                for j in range(2):
                    rcb_ps = pjp.tile([P, 512], FP32, tag="acc",
                                      name=f"rcb{sp}_{h}_{j}")
                    nc.tensor.matmul(
                        rcb_ps[0:Dh, :], lhsT=onesf64,
                        rhs=rrow[:, j * 512:(j + 1) * 512],
                        start=True, stop=True,
                    )
                    ot = outp.tile([Dh, 512], FP32, tag="outsb")
                    nc.vector.tensor_mul(
                        ot, cs[0:Dh, j * 512:(j + 1) * 512], rcb_ps[0:Dh, :])
                    nc.vector.tensor_scalar(
                        ot, ot,
                        bvT_sb[hi * Dh:(hi + 1) * Dh, hp:hp + 1], None,
                        OP.add,
                    )
                    # out DMA on the SWDGE queue: idle post-load, so its
                    # sem waits never back up the sync queue
                    nc.gpsimd.dma_start(
                        out=out[h, :, sp * 1024 + j * 512:
                                sp * 1024 + (j + 1) * 512], in_=ot)
            return run

        pending = None   # deferred phase-E back-chain of the previous block
        for h in range(HPC):             # heads sequential, uniform PE mode
            hp = h // 2
            for sp in range(2):          # s-half: columns sp*1024 .. +1024
                ctx_ps = [accp.tile([P, 512], FP32, tag="acc",
                                    name=f"ctx{sp}_{h}_{k}") for k in range(2)]
                for t in range(SC):
                    if t == 1 and pending is not None:
                        # previous block's normalize runs here so its PE
                        # outer-product never stalls the in-order PE queue
                        pending()
                        pending = None
                    for fn in dose.get((h, sp, t), ()):
                        fn()
                    sps = scp.tile([P, 1024], FP32, tag="sc")
                    for j in range(2):
                        s0 = sp * 1024 + j * 512
                        nc.tensor.matmul(
                            sps[:, j * 512:(j + 1) * 512],
                            lhsT=KTp[:, h, t * P:(t + 1) * P],
                            rhs=QT[:, hp, s0:s0 + 512],
                            start=True, stop=True,
                        )
                    et = etp.tile([P, 1024], BF16, tag="et")
                    nc.scalar.activation(et, sps, AF.Exp,
                                         bias=amask_sb[:, t:t + 1],
                                         scale=0.125)
                    for j in range(2):
                        nc.tensor.matmul(
                            ctx_ps[j][0:Dh + 1, :],
                            lhsT=Vaug[:, t, h, :],
                            rhs=et[:, j * 512:(j + 1) * 512],
                            start=(t == 0), stop=(t == SC - 1),
                        )
                # evacuate PSUM immediately so the ctx banks free early;
                # the rest of the normalize is deferred into the next block
                cs = rbp.tile([Dh + 1, 1024], FP32, tag="cs")
                for j in range(2):
                    nc.vector.tensor_copy(
                        cs[:, j * 512:(j + 1) * 512], ctx_ps[j][0:Dh + 1, :])
                pending = make_phase_e(h, sp, cs)
        pending()


def _build():
    key = "nc"
    if key in _CACHE:
        return _CACHE[key]
    nc = bacc.Bacc("TRN2", target_bir_lowering=False, debug=False,
                   enable_asserts=False)
    aps = {}

    def din(name, shape, dt):
        aps[name] = nc.dram_tensor(name, shape, dt, kind="ExternalInput").ap()

    din("xT", [DV, S], BF16)
    din("wq", [DV, E], BF16)
    din("wk", [DV, E], BF16)
    din("wv", [DV, E], BF16)
    din("wdq", [DT, E], BF16)
    din("wdk", [DT, E], BF16)
    din("txt", [T, DT], BF16)
    din("tmask", [T, 1], BF16)
    din("amask", [P, SC], FP32)
    din("bq", [P, ECH], FP32)
    din("bk", [P, ECH], FP32)
    din("bv", [P, ECH], FP32)
    din("bdq", [P, ECH], FP32)
    din("bdk", [P, ECH], FP32)
    aps["out"] = nc.dram_tensor("out", [HPC, Dh, S], FP32,
                                kind="ExternalOutput").ap()

    with tile.TileContext(nc) as tc:
        _emit(tc, aps)
    nc.compile()
    _CACHE[key] = nc
    return nc


def kernel(**inputs):
    global last_results
    hs = np.asarray(inputs["hidden_states"], dtype=np.float32)
    amask = np.asarray(inputs["attention_mask"], dtype=np.float32)
    txt = np.asarray(inputs["txt_embedding"], dtype=np.float32)
    tmask = np.asarray(inputs["txt_attention_mask"], dtype=np.float32)
    Wq = np.asarray(inputs["Wq"], dtype=np.float32)
    Wk = np.asarray(inputs["Wk"], dtype=np.float32)
    Wv = np.asarray(inputs["Wv"], dtype=np.float32)
    Wdq = np.asarray(inputs["Wdq"], dtype=np.float32)
    Wdk = np.asarray(inputs["Wdk"], dtype=np.float32)
    bq = np.asarray(inputs["bq"], dtype=np.float32)
    bk = np.asarray(inputs["bk"], dtype=np.float32)
    bv = np.asarray(inputs["bv"], dtype=np.float32)
    bdq = np.asarray(inputs["bdq"], dtype=np.float32)
    bdk = np.asarray(inputs["bdk"], dtype=np.float32)

    nc = _build()

    in_maps = []
    for c in range(NCORES):
        b, g = c // 2, c % 2
        cols = slice(g * E, (g + 1) * E)
        in_maps.append({
            "xT": np.ascontiguousarray(hs[b].T).astype(BF16_NP),
            "wq": Wq[:, cols].astype(BF16_NP),
            "wk": Wk[:, cols].astype(BF16_NP),
            "wv": Wv[:, cols].astype(BF16_NP),
            "wdq": Wdq[:, cols].astype(BF16_NP),
            "wdk": Wdk[:, cols].astype(BF16_NP),
            "txt": txt[b].astype(BF16_NP),
            "tmask": tmask[b].astype(BF16_NP),
            # pretransposed to [128, C] so each DMA is one contiguous
            # line per partition (per-element descriptors are pathological)
            "amask": np.ascontiguousarray(amask[b, 0, 0].reshape(SC, P).T),
            "bq": np.ascontiguousarray(bq[cols].reshape(ECH, P).T),
            "bk": np.ascontiguousarray(bk[cols].reshape(ECH, P).T),
            "bv": np.ascontiguousarray(bv[cols].reshape(ECH, P).T),
            "bdq": np.ascontiguousarray(bdq[cols].reshape(ECH, P).T),
            "bdk": np.ascontiguousarray(bdk[cols].reshape(ECH, P).T),
        })

    tr = int(os.environ.get("BASS_KERNEL_TRACE", "0"))
    if tr == 2:
        # warm the NEFF (compile+load+run untraced), then trace a second run
        run_bass_kernel_spmd(nc, in_maps, list(range(NCORES)), trace=False)
    res = run_bass_kernel_spmd(nc, in_maps, list(range(NCORES)), trace=bool(tr))
    last_results = res

    outp = np.empty((B, S, DV), dtype=np.float32)
    for c in range(NCORES):
        b, g = c // 2, c % 2
        # device output is ctx^T [head, d, s] -> [s, head*64+d]
        co = res.results[c]["out"].transpose(2, 0, 1).reshape(S, E)
        outp[b, :, g * E:(g + 1) * E] = co
    return outp


# revision 18
# speedup vs baseline: 1.2810x; 1.0786x over previous
"""BertImageSelfAttention Trainium2 kernel.

Shapes (fixed): hidden_states [4, 2048, 1024], 16 heads x 64, text [4, 64, 768].
Sharding: 8 cores = 4 batches x 2 head-groups (8 heads each). Each core computes
its batch's attention context for its 8 heads; host reassembles [4, 2048, 1024].

Per-core device pipeline (all matmuls bf16 with fp32 PSUM accumulation). The
scalar engine's Exp over the full S x S scores (33.5M elem/core at 1 elem/
cycle/lane) is the pacing engine, so the kernel is organized to saturate it:

  A. pooled text -> dynamic Q/K gates (tiny matmuls + sigmoid).
  B. loads: gate-critical tensors on the HWDGE queue first; x^T split into
     four s-slices on the SWDGE queue so projections start mid-load.
  C. Q/K projections for head-pair 0 run first; attention head 0 starts as
     soon as its first K/Q blocks land (~15us in). The V projection and the
     remaining Q/K e-chunks are dosed into the tensor engine's slack inside
     later heads' attention loops, underneath the Exp-bound steady state.
  D. per head: S^T tiles = K^T.T @ Q^T (keys on partitions), ACT Exp with
     scale=1/8 and per-partition bias=attention_mask, ctx^T[65,512] +=
     Vaug.T @ E^T over 16 key chunks (row 64 = softmax denominator).
  E. per (s-half, head): evict ctx^T to SBUF, DMA-gather the [1,1024]
     denominator row across 128 partitions, one cheap [128,8] reciprocal,
     DMA scatter+broadcast back to [64,1024], fused multiply + bias add,
     DMA out [64,1024] fp32 (output stays ctx^T; host transposes).
"""

import os

import numpy as np
import ml_dtypes

import concourse.bass as bass
import concourse.bacc as bacc
import concourse.tile as tile
from concourse import mybir
from concourse.bass_utils import run_bass_kernel_spmd

P = 128
B, S, DV = 4, 2048, 1024
H, Dh = 16, 64
T, DT = 64, 768
NCORES = 8
E = 512          # head-group width (8 heads x 64)
CC = DV // P     # 8 contraction chunks for projections
ECH = E // P     # 4 e-chunks
DC = DT // P     # 6 text-dim chunks
SC = S // P      # 16 seq chunks of 128
SBL = S // 512   # 4 seq blocks of 512
HPC = 8          # heads per core

FP32 = mybir.dt.float32
BF16 = mybir.dt.bfloat16
AF = mybir.ActivationFunctionType
OP = mybir.AluOpType

BF16_NP = ml_dtypes.bfloat16

_CACHE = {}

# module-level stash of the last BassKernelResults (for test.py introspection)
last_results = None


def _emit(tc, aps):
    nc = tc.nc
    xT = aps["xT"].rearrange("(c p) s -> p c s", p=P)          # [128, 8, 2048]
    wq = aps["wq"].rearrange("(c p) e -> p c e", p=P)          # [128, 8, 512]
    wk = aps["wk"].rearrange("(c p) e -> p c e", p=P)
    wv = aps["wv"].rearrange("(c p) e -> p c e", p=P)
    wdq = aps["wdq"].rearrange("(c p) e -> p c e", p=P)        # [128, 6, 512]
    wdk = aps["wdk"].rearrange("(c p) e -> p c e", p=P)
    txt = aps["txt"]                                           # [64, 768] bf16
    tmask = aps["tmask"]                                       # [64, 1] bf16
    amask = aps["amask"]                                       # [128, 16]
    bq = aps["bq"]                                             # [128, 4]
    bk = aps["bk"]
    bdq = aps["bdq"]
    bdk = aps["bdk"]
    bv = aps["bv"]                                             # [128, 4]
    out = aps["out"]                                           # [8, 64, 2048] f32

    from contextlib import ExitStack

    with ExitStack() as ctx:
        wpool = ctx.enter_context(tc.tile_pool(name="wpool", bufs=1))
        xpool = ctx.enter_context(tc.tile_pool(name="xpool", bufs=1))
        qkpool = ctx.enter_context(tc.tile_pool(name="qkpool", bufs=1))
        vpool = ctx.enter_context(tc.tile_pool(name="vpool", bufs=1))
        etp = ctx.enter_context(tc.tile_pool(name="etp", bufs=4))
        rbp = ctx.enter_context(tc.tile_pool(name="rbp", bufs=2))
        outp = ctx.enter_context(tc.tile_pool(name="outp", bufs=2))
        smallp = ctx.enter_context(tc.tile_pool(name="smallp", bufs=1))
        rcp = ctx.enter_context(tc.tile_pool(name="rcp", bufs=2))
        # PSUM: 8 banks = scp 2 x [128,1024] (2 banks each) + accp 2 x 1 bank
        # (ctx accumulators) + pjp 2 x 1 bank (projection scratch, dosed into
        # the attention loop — separate pool so rotation can't land on a
        # live ctx accumulator).
        accp = ctx.enter_context(tc.tile_pool(name="accp", bufs=2, space="PSUM"))
        pjp = ctx.enter_context(tc.tile_pool(name="pjp", bufs=2, space="PSUM"))
        scp = ctx.enter_context(tc.tile_pool(name="scp", bufs=2, space="PSUM"))

        # ---- gate-critical small loads (HWDGE queue) ----
        # text tensors padded to 128 partitions (zero rows 64..127) so every
        # matmul runs in uniform (128,128) PE tile mode — no mode switches.
        txt_sb = smallp.tile([P, DT], BF16, tag="txt")
        nc.vector.memset(txt_sb[T:P, :], 0.0)
        nc.sync.dma_start(out=txt_sb[0:T, :], in_=txt)
        # mask as a [128,128] stationary: column 0 = mask, rest zero -> M=128
        tmask_sb = smallp.tile([P, P], BF16, tag="tmask")
        nc.vector.memset(tmask_sb, 0.0)
        nc.sync.dma_start(out=tmask_sb[0:T, 0:1], in_=tmask)
        ones_sb = smallp.tile([P, 1], BF16, tag="ones")
        nc.vector.memset(ones_sb, 1.0)
        ones64b = smallp.tile([1, Dh], BF16, tag="ones64b")
        nc.vector.memset(ones64b, 1.0)
        bdq_sb = smallp.tile([P, ECH], FP32, tag="bdq")
        nc.sync.dma_start(out=bdq_sb, in_=bdq)
        bdk_sb = smallp.tile([P, ECH], FP32, tag="bdk")
        nc.sync.dma_start(out=bdk_sb, in_=bdk)
        bq_sb = smallp.tile([P, ECH], FP32, tag="bq")
        nc.sync.dma_start(out=bq_sb, in_=bq)
        bk_sb = smallp.tile([P, ECH], FP32, tag="bk")
        nc.sync.dma_start(out=bk_sb, in_=bk)

        wdq_sb = wpool.tile([P, DC, E], BF16, tag="wdq")
        nc.sync.dma_start(out=wdq_sb, in_=wdq)
        wdk_sb = wpool.tile([P, DC, E], BF16, tag="wdk")
        nc.sync.dma_start(out=wdk_sb, in_=wdk)
        wk_sb = wpool.tile([P, CC, E], BF16, tag="wk")
        nc.sync.dma_start(out=wk_sb, in_=wk)
        amask_sb = smallp.tile([P, SC], FP32, tag="amask")
        nc.sync.dma_start(out=amask_sb, in_=amask)
        bvT_sb = smallp.tile([P, ECH], FP32, tag="bvT")
        nc.sync.dma_start(out=bvT_sb, in_=bv)

        # ---- big loads, spread across three DMA queues ----
        # Per-queue bandwidth is ~125 GB/s, so the 8.6MB preload is split
        # evenly; each queue's order matches when its tensors are needed
        # (wq/xT_s0/s1 + gate weights first, wv/xT_s3 trail).
        xT_sb = xpool.tile([P, CC, S], BF16, tag="xT")
        wv_sb = wpool.tile([P, CC, E], BF16, tag="wv")
        wq_sb = wpool.tile([P, CC, E], BF16, tag="wq")
        nc.scalar.dma_start(out=wq_sb, in_=wq)
        nc.scalar.dma_start(out=wv_sb, in_=wv)
        nc.scalar.dma_start(out=xT_sb[:, :, 1536:2048], in_=xT[:, :, 1536:2048])
        nc.gpsimd.dma_start(out=xT_sb[:, :, 0:512], in_=xT[:, :, 0:512])
        nc.gpsimd.dma_start(out=xT_sb[:, :, 512:1024], in_=xT[:, :, 512:1024])
        nc.gpsimd.dma_start(out=xT_sb[:, :, 1024:1536], in_=xT[:, :, 1024:1536])

        # ---- phase A: pooled text + gates ----
        # poolT [128,6] (dt on partitions) computed DIRECTLY: col c =
        # txt_chunk_c.T @ tmask (no row-scatter DMAs). Col 6 row 0 holds
        # sum(mask); its reciprocal is partition-broadcast once and folded
        # into the sigmoid's per-partition scale (pool stays unnormalized).
        pps = pjp.tile([P, 512], FP32, tag="acc", name="poolps")
        for c in range(DC):
            nc.tensor.matmul(pps[:, c:c + 1],
                             lhsT=txt_sb[:, c * P:(c + 1) * P],
                             rhs=tmask_sb[:, 0:1], start=True, stop=True)
        nc.tensor.matmul(pps[:, DC:DC + 1], lhsT=tmask_sb, rhs=ones_sb,
                         start=True, stop=True)
        rmsum = smallp.tile([1, 1], FP32, tag="rmsum")
        nc.vector.reciprocal(rmsum, pps[0:1, DC:DC + 1])
        rmsumb = smallp.tile([P, 1], FP32, tag="rmsumb")
        rm_bcast = bass.AP(
            tensor=rmsum.tensor, offset=rmsum.offset,
            ap=[list(rmsum.ap[0]), [0, P]] + [list(d) for d in rmsum.ap[1:]],
        )
        nc.sync.dma_start(out=rmsumb, in_=rm_bcast)
        poolT = smallp.tile([P, DC], BF16, tag="poolT")
        nc.vector.tensor_copy(poolT, pps[:, 0:DC])

        # gates: g = 1 + sigmoid((pool_u @ Wd) * rmsum + bd); also g*b
        gq_sb = smallp.tile([P, ECH], FP32, tag="gq")
        gk_sb = smallp.tile([P, ECH], FP32, tag="gk")
        gbq_sb = smallp.tile([P, ECH], FP32, tag="gbq")
        gbk_sb = smallp.tile([P, ECH], FP32, tag="gbk")
        for (nm, wd_sb, bd_sb, b_sb, g_sb, gb_sb) in (
            ("q", wdq_sb, bdq_sb, bq_sb, gq_sb, gbq_sb),
            ("k", wdk_sb, bdk_sb, bk_sb, gk_sb, gbk_sb),
        ):
            for ec in range(ECH):
                gp = pjp.tile([P, 512], FP32, tag="acc", name=f"g{nm}{ec}")
                for c in range(DC):
                    nc.tensor.matmul(
                        gp[:, 0:1],
                        lhsT=wd_sb[:, c, ec * P:(ec + 1) * P],
                        rhs=poolT[:, c:c + 1],
                        start=(c == 0), stop=(c == DC - 1),
                    )
                nc.scalar.activation(g_sb[:, ec:ec + 1], gp[:, 0:1], AF.Sigmoid,
                                     bias=bd_sb[:, ec:ec + 1], scale=rmsumb)
            nc.vector.tensor_scalar(g_sb, g_sb, 1.0, None, OP.add)
            nc.vector.tensor_mul(gb_sb, g_sb, b_sb)

        # ---- projection emitters (dosed into the attention loop) ----
        QT = qkpool.tile([P, ECH, S], BF16, tag="QT")
        KTp = qkpool.tile([P, HPC, S], BF16, tag="KTp")
        nc.gpsimd.memset(KTp, 0.0)
        Vaug = vpool.tile([P, SC, HPC, Dh + 1], BF16, tag="Vaug")

        def emit_qk_half(ec, ss, which):
            """One projection half-group: 8 matmuls + eviction for Q or K."""
            sl = slice(ss * 512, (ss + 1) * 512)
            if which == "q":
                ps = pjp.tile([P, 512], FP32, tag="acc", name=f"psq{ec}_{ss}")
                for c in range(CC):
                    nc.tensor.matmul(
                        ps,
                        lhsT=wq_sb[:, c, ec * P:(ec + 1) * P],
                        rhs=xT_sb[:, c, sl],
                        start=(c == 0), stop=(c == CC - 1),
                    )
                # (x@W)*g + g*b fused into eviction, cast bf16
                nc.vector.tensor_scalar(
                    QT[:, ec, sl], ps,
                    gq_sb[:, ec:ec + 1], gbq_sb[:, ec:ec + 1],
                    OP.mult, OP.add,
                )
            else:
                psk = pjp.tile([P, 512], FP32, tag="acc", name=f"psk{ec}_{ss}")
                for c in range(CC):
                    nc.tensor.matmul(
                        psk,
                        lhsT=wk_sb[:, c, ec * P:(ec + 1) * P],
                        rhs=xT_sb[:, c, sl],
                        start=(c == 0), stop=(c == CC - 1),
                    )
                # K^T per-head zero-padded to 128 partitions (head h real on
                # partitions (h%2)*64..) so score matmuls contract K=128 in
                # the same (128,128) mode as everything else.
                for hi in range(2):
                    pp = slice(hi * Dh, (hi + 1) * Dh)
                    nc.vector.tensor_scalar(
                        KTp[pp, 2 * ec + hi, sl], psk[pp, :],
                        gk_sb[pp, ec:ec + 1], gbk_sb[pp, ec:ec + 1],
                        OP.mult, OP.add,
                    )

        def emit_v(t):
            """V projection t-chunk: 8 matmuls + eviction into Vaug."""
            ps = pjp.tile([P, 512], FP32, tag="acc", name=f"psv{t}")
            for c in range(CC):
                nc.tensor.matmul(
                    ps,
                    lhsT=xT_sb[:, c, t * P:(t + 1) * P],
                    rhs=wv_sb[:, c, :],
                    start=(c == 0), stop=(c == CC - 1),
                )
            nc.vector.tensor_copy(
                Vaug[:, t, :, 0:Dh],
                ps.rearrange("p (h d) -> p h d", h=HPC),
            )
            nc.vector.memset(Vaug[:, t, :, Dh:Dh + 1], 1.0)

        # head-pair 0's projections run up front, ordered so the first
        # score tile's inputs (Q ss0, Q ss1, K ss0) complete first.
        for ss, which in ((0, "q"), (1, "q"), (0, "k"), (1, "k"),
                          (2, "k"), (3, "k"), (2, "q"), (3, "q")):
            emit_qk_half(0, ss, which)

        # Remaining projections are dosed into attention t-iterations:
        # dose[(h, sp, t)] = emitters to run before that tile's score
        # matmuls. h0/sp0 hosts V (ctx t needs Vaug[t] that same iter);
        # h1, h3, h5 host the ec1..ec3 half-groups, one per 4 iters.
        dose = {}
        for t in range(SC):
            dose[(0, 0, t)] = [lambda t=t: emit_v(t)]
        for ec in (1, 2, 3):
            h = 2 * ec - 1
            for k in range(2 * SBL):
                sp, t = divmod(4 * k + 3, SC)
                ss, which = k // 2, ("q", "k")[k % 2]
                dose.setdefault((h, sp, t), []).append(
                    lambda ec=ec, ss=ss, w=which: emit_qk_half(ec, ss, w))

        # ---- phases D+E: attention ----
        # output stays in ctx^T layout [head, d, s]; host transposes to [s, e]
        def make_phase_e(h, sp, cs):
            hp, hi = h // 2, h % 2

            def run():
                # denominator row [1,1024] -> [128,8] across partitions, one
                # cheap reciprocal, scatter back to a row, then partition-
                # broadcast via a tiny PE outer product (ones[64] x rrow).
                rpack = rcp.tile([P, 8], FP32, tag="rpack")
                nc.sync.dma_start(out=rpack, in_=cs[Dh:Dh + 1, :])
                # bf16 reciprocal: keeps the broadcast outer product in the
                # PE's bf16 pipeline (an fp32 LOW_HIGH matmul mode-switch
                # slows the next ~10 matmuls by ~65%)
                rpb = rcp.tile([P, 8], BF16, tag="rpb")
                with nc.allow_low_precision(reason="bf16 softmax denom recip"):
                    nc.vector.reciprocal(rpb, rpack)
                rrow = rcp.tile([1, 1024], BF16, tag="rrow")
                nc.sync.dma_start(out=rrow, in_=rpb)
                for j in range(2):
                    rcb_ps = pjp.tile([P, 512], FP32, tag="acc",
                                      name=f"rcb{sp}_{h}_{j}")
                    nc.tensor.matmul(
                        rcb_ps[0:Dh, :], lhsT=ones64b,
                        rhs=rrow[:, j * 512:(j + 1) * 512],
                        start=True, stop=True,
                    )
                    ot = outp.tile([Dh, 512], FP32, tag="outsb")
                    nc.vector.tensor_mul(
                        ot, cs[0:Dh, j * 512:(j + 1) * 512], rcb_ps[0:Dh, :])
                    nc.vector.tensor_scalar(
                        ot, ot,
                        bvT_sb[hi * Dh:(hi + 1) * Dh, hp:hp + 1], None,
                        OP.add,
                    )
                    # out DMA on the SWDGE queue: idle post-load, so its
                    # sem waits never back up the sync queue
                    nc.gpsimd.dma_start(
                        out=out[h, :, sp * 1024 + j * 512:
                                sp * 1024 + (j + 1) * 512], in_=ot)
            return run

        pending = None   # deferred phase-E back-chain of the previous block
        for h in range(HPC):             # heads sequential, uniform PE mode
            hp = h // 2
            for sp in range(2):          # s-half: columns sp*1024 .. +1024
                ctx_ps = [accp.tile([P, 512], FP32, tag="acc",
                                    name=f"ctx{sp}_{h}_{k}") for k in range(2)]
                for t in range(SC):
                    if t == 1 and pending is not None:
                        # previous block's normalize runs here so its PE
                        # outer-product never stalls the in-order PE queue
                        pending()
                        pending = None
                    for fn in dose.get((h, sp, t), ()):
                        fn()
                    sps = scp.tile([P, 1024], FP32, tag="sc")
                    for j in range(2):
                        s0 = sp * 1024 + j * 512
                        nc.tensor.matmul(
                            sps[:, j * 512:(j + 1) * 512],
                            lhsT=KTp[:, h, t * P:(t + 1) * P],
                            rhs=QT[:, hp, s0:s0 + 512],
                            start=True, stop=True,
                        )
                    et = etp.tile([P, 1024], BF16, tag="et")
                    nc.scalar.activation(et, sps, AF.Exp,
                                         bias=amask_sb[:, t:t + 1],
                                         scale=0.125)
                    for j in range(2):
                        nc.tensor.matmul(
                            ctx_ps[j][0:Dh + 1, :],
                            lhsT=Vaug[:, t, h, :],
                            rhs=et[:, j * 512:(j + 1) * 512],
                            start=(t == 0), stop=(t == SC - 1),
                        )
                # evacuate PSUM immediately so the ctx banks free early;
                # the rest of the normalize is deferred into the next block
                cs = rbp.tile([Dh + 1, 1024], FP32, tag="cs")
                for j in range(2):
                    nc.vector.tensor_copy(
                        cs[:, j * 512:(j + 1) * 512], ctx_ps[j][0:Dh + 1, :])
                pending = make_phase_e(h, sp, cs)
        pending()


def _build():
    key = "nc"
    if key in _CACHE:
        return _CACHE[key]
    nc = bacc.Bacc("TRN2", target_bir_lowering=False, debug=False,
                   enable_asserts=False)
    aps = {}

    def din(name, shape, dt):
        aps[name] = nc.dram_tensor(name, shape, dt, kind="ExternalInput").ap()

    din("xT", [DV, S], BF16)
    din("wq", [DV, E], BF16)
    din("wk", [DV, E], BF16)
    din("wv", [DV, E], BF16)
    din("wdq", [DT, E], BF16)
    din("wdk", [DT, E], BF16)
    din("txt", [T, DT], BF16)
    din("tmask", [T, 1], BF16)
    din("amask", [P, SC], FP32)
    din("bq", [P, ECH], FP32)
    din("bk", [P, ECH], FP32)
    din("bv", [P, ECH], FP32)
    din("bdq", [P, ECH], FP32)
    din("bdk", [P, ECH], FP32)
    aps["out"] = nc.dram_tensor("out", [HPC, Dh, S], FP32,
                                kind="ExternalOutput").ap()

    with tile.TileContext(nc) as tc:
        _emit(tc, aps)
    nc.compile()
    _CACHE[key] = nc
    return nc


def kernel(**inputs):
    global last_results
    hs = np.asarray(inputs["hidden_states"], dtype=np.float32)
    amask = np.asarray(inputs["attention_mask"], dtype=np.float32)
    txt = np.asarray(inputs["txt_embedding"], dtype=np.float32)
    tmask = np.asarray(inputs["txt_attention_mask"], dtype=np.float32)
    Wq = np.asarray(inputs["Wq"], dtype=np.float32)
    Wk = np.asarray(inputs["Wk"], dtype=np.float32)
    Wv = np.asarray(inputs["Wv"], dtype=np.float32)
    Wdq = np.asarray(inputs["Wdq"], dtype=np.float32)
    Wdk = np.asarray(inputs["Wdk"], dtype=np.float32)
    bq = np.asarray(inputs["bq"], dtype=np.float32)
    bk = np.asarray(inputs["bk"], dtype=np.float32)
    bv = np.asarray(inputs["bv"], dtype=np.float32)
    bdq = np.asarray(inputs["bdq"], dtype=np.float32)
    bdk = np.asarray(inputs["bdk"], dtype=np.float32)

    nc = _build()

    in_maps = []
    for c in range(NCORES):
        b, g = c // 2, c % 2
        cols = slice(g * E, (g + 1) * E)
        in_maps.append({
            "xT": np.ascontiguousarray(hs[b].T).astype(BF16_NP),
            "wq": Wq[:, cols].astype(BF16_NP),
            "wk": Wk[:, cols].astype(BF16_NP),
            "wv": Wv[:, cols].astype(BF16_NP),
            "wdq": Wdq[:, cols].astype(BF16_NP),
            "wdk": Wdk[:, cols].astype(BF16_NP),
            "txt": txt[b].astype(BF16_NP),
            "tmask": tmask[b].astype(BF16_NP),
            # pretransposed to [128, C] so each DMA is one contiguous
            # line per partition (per-element descriptors are pathological)
            "amask": np.ascontiguousarray(amask[b, 0, 0].reshape(SC, P).T),
            "bq": np.ascontiguousarray(bq[cols].reshape(ECH, P).T),
            "bk": np.ascontiguousarray(bk[cols].reshape(ECH, P).T),
            "bv": np.ascontiguousarray(bv[cols].reshape(ECH, P).T),
            "bdq": np.ascontiguousarray(bdq[cols].reshape(ECH, P).T),
            "bdk": np.ascontiguousarray(bdk[cols].reshape(ECH, P).T),
        })

    tr = int(os.environ.get("BASS_KERNEL_TRACE", "0"))
    if tr == 2:
        # warm the NEFF (compile+load+run untraced), then trace a second run
        run_bass_kernel_spmd(nc, in_maps, list(range(NCORES)), trace=False)
    res = run_bass_kernel_spmd(nc, in_maps, list(range(NCORES)), trace=bool(tr))
    last_results = res

    outp = np.empty((B, S, DV), dtype=np.float32)
    for c in range(NCORES):
        b, g = c // 2, c % 2
        # device output is ctx^T [head, d, s] -> [s, head*64+d]
        co = res.results[c]["out"].transpose(2, 0, 1).reshape(S, E)
        outp[b, :, g * E:(g + 1) * E] = co
    return outp


# revision 32
# speedup vs baseline: 1.3960x; 1.0897x over previous
"""BertImageSelfAttention Trainium2 kernel.

Shapes (fixed): hidden_states [4, 2048, 1024], 16 heads x 64, text [4, 64, 768].
Sharding: 8 cores = 4 batches x 2 head-groups (8 heads each). Each core computes
its batch's attention context for its 8 heads; host reassembles [4, 2048, 1024].

Per-core pipeline (matmuls bf16, fp32 PSUM). The scalar engine's Exp over the
full S x S scores (33.5M elem/core at 1 elem/cycle/lane, ~285us) and the tensor
engine's streaming (~1430 N=512 matmuls, ~310us) are a dual roofline; the
kernel is organized to keep both saturated from ~20us in:

  A. poolT (text pool, dt-on-partitions) built directly by 6 tiny matmuls
     (txt^T @ tmask) — no row-scatter DMAs; 1/sum(mask) folded into the
     sigmoid's per-partition scale. Both gates fold into Q (scores =
     (xWk+bk) . [gq*gk*(xWq+bq)]), so K evictions are gate-free.
  B. loads spread over three DMA queues (~125 GB/s each), ordered by need;
     1-D tensors are host-pretransposed to [128, C] (per-element-descriptor
     DMAs are pathological). Only Q ss0/ss1 + K ss0 projections run before
     attention; V and all other Q/K half-groups are dosed into attention
     t-iterations at reduced scheduler priority, filling PE wait slots.
  C. per head: S^T tiles = K^T.T @ Q^T (keys on partitions), one ACT Exp per
     [128,1024] PSUM tile (scale=1/8, bias=attention_mask), ctx^T[65,512] +=
     Vaug.T @ E^T over 16 key chunks (row 64 = softmax denominator).
  D. normalize per (s-half, head), deferred into the next block's t-loop:
     evict ctx^T, DMA-gather the [1,1024] denom row to [128,8], one cheap
     bf16 reciprocal, scatter to a row, partition-broadcast via a tiny bf16
     PE outer product (fp32 matmuls would mode-switch the PE), fused
     multiply + bias add, out DMA [64,512] fp32 on the idle SWDGE queue.
     Output stays ctx^T [head, d, s]; host transposes.
"""

import os

import numpy as np
import ml_dtypes

import concourse.bass as bass
import concourse.bacc as bacc
import concourse.tile as tile
from concourse import mybir
from concourse.bass_utils import run_bass_kernel_spmd

P = 128
B, S, DV = 4, 2048, 1024
H, Dh = 16, 64
T, DT = 64, 768
NCORES = 8
E = 512          # head-group width (8 heads x 64)
CC = DV // P     # 8 contraction chunks for projections
ECH = E // P     # 4 e-chunks
DC = DT // P     # 6 text-dim chunks
SC = S // P      # 16 seq chunks of 128
SBL = S // 512   # 4 seq blocks of 512
HPC = 8          # heads per core

FP32 = mybir.dt.float32
BF16 = mybir.dt.bfloat16
AF = mybir.ActivationFunctionType
OP = mybir.AluOpType

BF16_NP = ml_dtypes.bfloat16

_CACHE = {}

# module-level stash of the last BassKernelResults (for test.py introspection)
last_results = None


def _emit(tc, aps):
    nc = tc.nc
    xT = aps["xT"].rearrange("(c p) s -> p c s", p=P)          # [128, 8, 2048]
    wq = aps["wq"].rearrange("(c p) e -> p c e", p=P)          # [128, 8, 512]
    wk = aps["wk"].rearrange("(c p) e -> p c e", p=P)
    wv = aps["wv"].rearrange("(c p) e -> p c e", p=P)
    wdq = aps["wdq"].rearrange("(c p) e -> p c e", p=P)        # [128, 6, 512]
    wdk = aps["wdk"].rearrange("(c p) e -> p c e", p=P)
    txt = aps["txt"]                                           # [64, 768] bf16
    tmask = aps["tmask"]                                       # [64, 1] bf16
    smallpack = aps["smallpack"]                               # [128, 36]
    out = aps["out"]                                           # [8, 64, 2048] f32

    from contextlib import ExitStack

    with ExitStack() as ctx:
        wpool = ctx.enter_context(tc.tile_pool(name="wpool", bufs=1))
        xpool = ctx.enter_context(tc.tile_pool(name="xpool", bufs=1))
        qkpool = ctx.enter_context(tc.tile_pool(name="qkpool", bufs=1))
        vpool = ctx.enter_context(tc.tile_pool(name="vpool", bufs=1))
        etp = ctx.enter_context(tc.tile_pool(name="etp", bufs=8))
        rbp = ctx.enter_context(tc.tile_pool(name="rbp", bufs=2))
        outp = ctx.enter_context(tc.tile_pool(name="outp", bufs=2))
        smallp = ctx.enter_context(tc.tile_pool(name="smallp", bufs=1))
        rcp = ctx.enter_context(tc.tile_pool(name="rcp", bufs=2))
        # PSUM: 8 banks = scp 2 x [128,1024] (2 banks each) + accp 2 x 1 bank
        # (ctx accumulators) + pjp 2 x 1 bank (projection scratch, dosed into
        # the attention loop — separate pool so rotation can't land on a
        # live ctx accumulator).
        accp = ctx.enter_context(tc.tile_pool(name="accp", bufs=2, space="PSUM"))
        pjp = ctx.enter_context(tc.tile_pool(name="pjp", bufs=2, space="PSUM"))
        scp = ctx.enter_context(tc.tile_pool(name="scp", bufs=2, space="PSUM"))

        # ---- gate-critical small loads (HWDGE queue) ----
        # text tensors padded to 128 partitions (zero rows 64..127) so every
        # matmul runs in uniform (128,128) PE tile mode — no mode switches.
        txt_sb = smallp.tile([P, DT], BF16, tag="txt")
        nc.vector.memset(txt_sb[T:P, :], 0.0)
        nc.scalar.dma_start(out=txt_sb[0:T, :], in_=txt)
        # mask as a [128,128] stationary: column 0 = mask, rest zero -> M=128
        tmask_sb = smallp.tile([P, P], BF16, tag="tmask")
        nc.vector.memset(tmask_sb, 0.0)
        nc.scalar.dma_start(out=tmask_sb[0:T, 0:1], in_=tmask)
        ones_sb = smallp.tile([P, 1], BF16, tag="ones")
        nc.vector.memset(ones_sb, 1.0)
        ones64b = smallp.tile([1, Dh], BF16, tag="ones64b")
        nc.vector.memset(ones64b, 1.0)
        spk = smallp.tile([P, SC + 5 * ECH], FP32, tag="spk")
        nc.scalar.dma_start(out=spk, in_=smallpack)
        amask_sb = spk[:, 0:SC]
        bq_sb = spk[:, SC:SC + ECH]
        bk_sb = spk[:, SC + ECH:SC + 2 * ECH]
        bdq_sb = spk[:, SC + 2 * ECH:SC + 3 * ECH]
        bdk_sb = spk[:, SC + 3 * ECH:SC + 4 * ECH]
        bvT_sb = spk[:, SC + 4 * ECH:SC + 5 * ECH]

        wk_sb = wpool.tile([P, CC, E], BF16, tag="wk")
        nc.sync.dma_start(out=wk_sb, in_=wk)

        # ---- big loads, spread across three DMA queues ----
        # Per-queue bandwidth is ~125 GB/s, so the 8.6MB preload is split
        # evenly; each queue's order matches when its tensors are needed
        # (wq/xT_s0/s1 + gate weights first, wv/xT_s3 trail).
        xT_sb = xpool.tile([P, CC, S], BF16, tag="xT")
        wv_sb = wpool.tile([P, CC, E], BF16, tag="wv")
        wq_sb = wpool.tile([P, CC, E], BF16, tag="wq")
        wdk_sb = wpool.tile([P, DC, E], BF16, tag="wdk")
        nc.scalar.dma_start(out=wdk_sb, in_=wdk)
        wdq_sb = wpool.tile([P, DC, E], BF16, tag="wdq")
        nc.scalar.dma_start(out=wdq_sb, in_=wdq)
        nc.scalar.dma_start(out=wq_sb, in_=wq)
        for ss in range(SBL):
            sl = slice(ss * 512, (ss + 1) * 512)
            nc.gpsimd.dma_start(out=xT_sb[:, :, sl], in_=xT[:, :, sl])

        # ---- phase A: pooled text + gates ----
        # poolT [128,6] (dt on partitions) computed DIRECTLY: col c =
        # txt_chunk_c.T @ tmask (no row-scatter DMAs). Col 6 row 0 holds
        # sum(mask); its reciprocal is partition-broadcast once and folded
        # into the sigmoid's per-partition scale (pool stays unnormalized).
        pps = pjp.tile([P, 512], FP32, tag="acc", name="poolps")
        for c in range(DC):
            nc.tensor.matmul(pps[:, c:c + 1],
                             lhsT=txt_sb[:, c * P:(c + 1) * P],
                             rhs=tmask_sb[:, 0:1], start=True, stop=True)
        nc.tensor.matmul(pps[:, DC:DC + 1], lhsT=tmask_sb, rhs=ones_sb,
                         start=True, stop=True)
        rmsum = smallp.tile([1, 1], FP32, tag="rmsum")
        nc.vector.reciprocal(rmsum, pps[0:1, DC:DC + 1])
        rmsumb = smallp.tile([P, 1], FP32, tag="rmsumb")
        rm_bcast = bass.AP(
            tensor=rmsum.tensor, offset=rmsum.offset,
            ap=[list(rmsum.ap[0]), [0, P]] + [list(d) for d in rmsum.ap[1:]],
        )
        nc.sync.dma_start(out=rmsumb, in_=rm_bcast)
        poolT = smallp.tile([P, DC], BF16, tag="poolT")
        nc.vector.tensor_copy(poolT, pps[:, 0:DC])

        # gates: g = 1 + sigmoid((pool_u @ Wd) * rmsum + bd); also g*b
        gq_sb = smallp.tile([P, ECH], FP32, tag="gq")
        gk_sb = smallp.tile([P, ECH], FP32, tag="gk")
        gbq_sb = smallp.tile([P, ECH], FP32, tag="gbq")
        gbk_sb = smallp.tile([P, ECH], FP32, tag="gbk")
        for (nm, wd_sb, bd_sb, b_sb, g_sb, gb_sb) in (
            ("q", wdq_sb, bdq_sb, bq_sb, gq_sb, gbq_sb),
            ("k", wdk_sb, bdk_sb, bk_sb, gk_sb, gbk_sb),
        ):
            for ec in range(ECH):
                gp = pjp.tile([P, 512], FP32, tag="acc", name=f"g{nm}{ec}")
                for c in range(DC):
                    nc.tensor.matmul(
                        gp[:, 0:1],
                        lhsT=wd_sb[:, c, ec * P:(ec + 1) * P],
                        rhs=poolT[:, c:c + 1],
                        start=(c == 0), stop=(c == DC - 1),
                    )
                nc.scalar.activation(g_sb[:, ec:ec + 1], gp[:, 0:1], AF.Sigmoid,
                                     bias=bd_sb[:, ec:ec + 1], scale=rmsumb)
            nc.vector.tensor_scalar(g_sb, g_sb, 1.0, None, OP.add)
            nc.vector.tensor_mul(gb_sb, g_sb, b_sb)
        # fold BOTH gates into Q: scores = (xWk+bk) . [gqk*(xWq+bq)] with
        # gqk = gq*gk — the K eviction becomes gate-free (starts earlier)
        gqk_sb = smallp.tile([P, ECH], FP32, tag="gqk")
        nc.vector.tensor_mul(gqk_sb, gq_sb, gk_sb)
        gqkbq_sb = smallp.tile([P, ECH], FP32, tag="gqkbq")
        nc.vector.tensor_mul(gqkbq_sb, gqk_sb, bq_sb)
        nc.scalar.dma_start(out=wv_sb, in_=wv)

        # ---- projection emitters (dosed into the attention loop) ----
        QT = qkpool.tile([P, ECH, S], BF16, tag="QT")
        KTp = qkpool.tile([P, HPC, S], BF16, tag="KTp")
        nc.gpsimd.memset(KTp, 0.0)
        Vaug = vpool.tile([P, SC, HPC, Dh + 1], BF16, tag="Vaug")

        def emit_qk_half(ec, ss, which):
            """One projection half-group: 8 matmuls + eviction for Q or K."""
            sl = slice(ss * 512, (ss + 1) * 512)
            if which == "q":
                ps = pjp.tile([P, 512], FP32, tag="acc", name=f"psq{ec}_{ss}")
                for c in range(CC):
                    nc.tensor.matmul(
                        ps,
                        lhsT=wq_sb[:, c, ec * P:(ec + 1) * P],
                        rhs=xT_sb[:, c, sl],
                        start=(c == 0), stop=(c == CC - 1),
                    )
                # (x@W)*gqk + gqk*bq fused into eviction, cast bf16
                nc.vector.tensor_scalar(
                    QT[:, ec, sl], ps,
                    gqk_sb[:, ec:ec + 1], gqkbq_sb[:, ec:ec + 1],
                    OP.mult, OP.add,
                )
            else:
                psk = pjp.tile([P, 512], FP32, tag="acc", name=f"psk{ec}_{ss}")
                for c in range(CC):
                    nc.tensor.matmul(
                        psk,
                        lhsT=wk_sb[:, c, ec * P:(ec + 1) * P],
                        rhs=xT_sb[:, c, sl],
                        start=(c == 0), stop=(c == CC - 1),
                    )
                # K^T per-head zero-padded to 128 partitions (head h real on
                # partitions (h%2)*64..) so score matmuls contract K=128 in
                # the same (128,128) mode as everything else.
                for hi in range(2):
                    pp = slice(hi * Dh, (hi + 1) * Dh)
                    nc.vector.tensor_scalar(
                        KTp[pp, 2 * ec + hi, sl], psk[pp, :],
                        bk_sb[pp, ec:ec + 1], None,
                        OP.add,
                    )

        def emit_v(t):
            """V projection t-chunk: 8 matmuls + eviction into Vaug."""
            ps = pjp.tile([P, 512], FP32, tag="acc", name=f"psv{t}")
            for c in range(CC):
                nc.tensor.matmul(
                    ps,
                    lhsT=xT_sb[:, c, t * P:(t + 1) * P],
                    rhs=wv_sb[:, c, :],
                    start=(c == 0), stop=(c == CC - 1),
                )
            nc.vector.tensor_copy(
                Vaug[:, t, :, 0:Dh],
                ps.rearrange("p (h d) -> p h d", h=HPC),
            )
            nc.vector.memset(Vaug[:, t, :, Dh:Dh + 1], 1.0)

        # Only the first score tile's inputs (Q ss0, Q ss1, K ss0) run up
        # front; every other projection is dosed into the attention loop at
        # reduced priority, where it fills tensor-engine wait slots.
        for ss, which in ((0, "q"), (1, "q"), (0, "k")):
            emit_qk_half(0, ss, which)

        # Remaining projections are dosed into attention t-iterations:
        # dose[(h, sp, t)] = emitters to run before that tile's score
        # matmuls. h0/sp0 hosts V[8..15] (ctx t needs Vaug[t] that same
        # iter); ec1 spreads over h0/sp1 + h1, ec2 over h2+h3, ec3 over
        # h4+h5 — each finishing before the consuming head pair starts.
        dose = {}
        for t in range(SC):
            dose[(0, 0, t)] = [lambda t=t: emit_v(t)]
        for k, (ss, which) in enumerate(
                ((1, "k"), (2, "k"), (3, "k"), (2, "q"), (3, "q"))):
            dose[(0, 0, 2 * k + 1)].append(
                lambda ss=ss, w=which: emit_qk_half(0, ss, w))
        for ec in (1, 2, 3):
            for k in range(2 * SBL):
                if ec == 1:
                    g = 8 * k + 3          # iters 3..59 of h0sp1+h1
                    h = (48 + g) // 32     # h0 (sp1) for g<16, else h1
                    sp, t = divmod((16 + g) % 32, SC)
                else:
                    g = 8 * k + 3
                    h = 2 * (ec - 1) + g // 32
                    sp, t = divmod(g % 32, SC)
                ss, which = k // 2, ("q", "k")[k % 2]
                dose.setdefault((h, sp, t), []).append(
                    lambda ec=ec, ss=ss, w=which: emit_qk_half(ec, ss, w))

        # ---- phases D+E: attention ----
        # output stays in ctx^T layout [head, d, s]; host transposes to [s, e]
        def make_phase_e(h, sp, cs):
            hp, hi = h // 2, h % 2

            def run():
                # denominator row [1,1024] -> [128,8] across partitions, one
                # cheap reciprocal, scatter back to a row, then partition-
                # broadcast via a tiny PE outer product (ones[64] x rrow).
                rpack = rcp.tile([P, 8], FP32, tag="rpack")
                nc.sync.dma_start(out=rpack, in_=cs[Dh:Dh + 1, :])
                # bf16 reciprocal: keeps the broadcast outer product in the
                # PE's bf16 pipeline (an fp32 LOW_HIGH matmul mode-switch
                # slows the next ~10 matmuls by ~65%)
                rpb = rcp.tile([P, 8], BF16, tag="rpb")
                with nc.allow_low_precision(reason="bf16 softmax denom recip"):
                    nc.vector.reciprocal(rpb, rpack)
                rrow = rcp.tile([1, 1024], BF16, tag="rrow")
                nc.sync.dma_start(out=rrow, in_=rpb)
                for j in range(2):
                    rcb_ps = pjp.tile([P, 512], FP32, tag="acc",
                                      name=f"rcb{sp}_{h}_{j}")
                    nc.tensor.matmul(
                        rcb_ps[0:Dh, :], lhsT=ones64b,
                        rhs=rrow[:, j * 512:(j + 1) * 512],
                        start=True, stop=True,
                    )
                    ot = outp.tile([Dh, 512], FP32, tag="outsb")
                    nc.vector.tensor_mul(
                        ot, cs[0:Dh, j * 512:(j + 1) * 512], rcb_ps[0:Dh, :])
                    nc.vector.tensor_scalar(
                        ot, ot,
                        bvT_sb[hi * Dh:(hi + 1) * Dh, hp:hp + 1], None,
                        OP.add,
                    )
                    # out DMA on the SWDGE queue: idle post-load, so its
                    # sem waits never back up the sync queue
                    nc.gpsimd.dma_start(
                        out=out[h, :, sp * 1024 + j * 512:
                                sp * 1024 + (j + 1) * 512], in_=ot)
            return run

        pending = None   # deferred phase-E back-chain of the previous block
        for h in range(HPC):             # heads sequential, uniform PE mode
            hp = h // 2
            for sp in range(2):          # s-half: columns sp*1024 .. +1024
                ctx_ps = [accp.tile([P, 512], FP32, tag="acc",
                                    name=f"ctx{sp}_{h}_{k}") for k in range(2)]
                for t in range(SC):
                    if t == 1 and pending is not None:
                        # previous block's normalize runs here, at reduced
                        # scheduler priority so its PE outer-product slots
                        # into PE-wait gaps instead of blocking the queue
                        with tc.high_priority(offset=-80):
                            pending()
                        pending = None
                    if (h, sp, t) in dose:
                        with tc.high_priority(offset=-80):
                            for fn in dose[(h, sp, t)]:
                                fn()
                    sps = scp.tile([P, 1024], FP32, tag="sc")
                    for j in range(2):
                        s0 = sp * 1024 + j * 512
                        nc.tensor.matmul(
                            sps[:, j * 512:(j + 1) * 512],
                            lhsT=KTp[:, h, t * P:(t + 1) * P],
                            rhs=QT[:, hp, s0:s0 + 512],
                            start=True, stop=True,
                        )
                    et = etp.tile([P, 1024], BF16, tag="et")
                    nc.scalar.activation(et, sps, AF.Exp,
                                         bias=amask_sb[:, t:t + 1],
                                         scale=0.125)
                    for j in range(2):
                        nc.tensor.matmul(
                            ctx_ps[j][0:Dh + 1, :],
                            lhsT=Vaug[:, t, h, :],
                            rhs=et[:, j * 512:(j + 1) * 512],
                            start=(t == 0), stop=(t == SC - 1),
                        )
                # evacuate PSUM immediately so the ctx banks free early;
                # the rest of the normalize is deferred into the next block
                cs = rbp.tile([Dh + 1, 1024], FP32, tag="cs")
                for j in range(2):
                    nc.vector.tensor_copy(
                        cs[:, j * 512:(j + 1) * 512], ctx_ps[j][0:Dh + 1, :])
                pending = make_phase_e(h, sp, cs)
        pending()


def _build():
    key = "nc"
    if key in _CACHE:
        return _CACHE[key]
    nc = bacc.Bacc("TRN2", target_bir_lowering=False, debug=False,
                   enable_asserts=False)
    aps = {}

    def din(name, shape, dt):
        aps[name] = nc.dram_tensor(name, shape, dt, kind="ExternalInput").ap()

    din("xT", [DV, S], BF16)
    din("wq", [DV, E], BF16)
    din("wk", [DV, E], BF16)
    din("wv", [DV, E], BF16)
    din("wdq", [DT, E], BF16)
    din("wdk", [DT, E], BF16)
    din("txt", [T, DT], BF16)
    din("tmask", [T, 1], BF16)
    # amask[0:16] | bq[16:20] | bk[20:24] | bdq[24:28] | bdk[28:32] | bv[32:36]
    din("smallpack", [P, SC + 5 * ECH], FP32)
    aps["out"] = nc.dram_tensor("out", [HPC, Dh, S], FP32,
                                kind="ExternalOutput").ap()

    with tile.TileContext(nc) as tc:
        _emit(tc, aps)
    nc.compile()
    _CACHE[key] = nc
    return nc


def kernel(**inputs):
    global last_results
    hs = np.asarray(inputs["hidden_states"], dtype=np.float32)
    amask = np.asarray(inputs["attention_mask"], dtype=np.float32)
    txt = np.asarray(inputs["txt_embedding"], dtype=np.float32)
    tmask = np.asarray(inputs["txt_attention_mask"], dtype=np.float32)
    Wq = np.asarray(inputs["Wq"], dtype=np.float32)
    Wk = np.asarray(inputs["Wk"], dtype=np.float32)
    Wv = np.asarray(inputs["Wv"], dtype=np.float32)
    Wdq = np.asarray(inputs["Wdq"], dtype=np.float32)
    Wdk = np.asarray(inputs["Wdk"], dtype=np.float32)
    bq = np.asarray(inputs["bq"], dtype=np.float32)
    bk = np.asarray(inputs["bk"], dtype=np.float32)
    bv = np.asarray(inputs["bv"], dtype=np.float32)
    bdq = np.asarray(inputs["bdq"], dtype=np.float32)
    bdk = np.asarray(inputs["bdk"], dtype=np.float32)

    nc = _build()

    in_maps = []
    for c in range(NCORES):
        b, g = c // 2, c % 2
        cols = slice(g * E, (g + 1) * E)
        in_maps.append({
            "xT": np.ascontiguousarray(hs[b].T).astype(BF16_NP),
            "wq": Wq[:, cols].astype(BF16_NP),
            "wk": Wk[:, cols].astype(BF16_NP),
            "wv": Wv[:, cols].astype(BF16_NP),
            "wdq": Wdq[:, cols].astype(BF16_NP),
            "wdk": Wdk[:, cols].astype(BF16_NP),
            "txt": txt[b].astype(BF16_NP),
            "tmask": tmask[b].astype(BF16_NP),
            # all 1-D tensors pretransposed to [128, C] and packed into ONE
            # DMA (per-element descriptors and per-transfer queue latency
            # are both pathological for small loads)
            "smallpack": np.ascontiguousarray(np.concatenate([
                amask[b, 0, 0].reshape(SC, P).T,
                bq[cols].reshape(ECH, P).T,
                bk[cols].reshape(ECH, P).T,
                bdq[cols].reshape(ECH, P).T,
                bdk[cols].reshape(ECH, P).T,
                bv[cols].reshape(ECH, P).T,
            ], axis=1)),
        })

    tr = int(os.environ.get("BASS_KERNEL_TRACE", "0"))
    if tr == 2:
        # warm the NEFF (compile+load+run untraced), then trace a second run
        run_bass_kernel_spmd(nc, in_maps, list(range(NCORES)), trace=False)
    res = run_bass_kernel_spmd(nc, in_maps, list(range(NCORES)), trace=bool(tr))
    last_results = res

    outp = np.empty((B, S, DV), dtype=np.float32)
    for c in range(NCORES):
        b, g = c // 2, c % 2
        # device output is ctx^T [head, d, s] -> [s, head*64+d]
        co = res.results[c]["out"].transpose(2, 0, 1).reshape(S, E)
        outp[b, :, g * E:(g + 1) * E] = co
    return outp
